# revision 35
# baseline (speedup 1.0000x reference)
"""Trainium2 Bass kernel for a 3-layer BiLSTM + ReLU + residual + LayerNorm.

V5: device computes only the 3-layer BiLSTM; final stage (ReLU + residual +
LayerNorm + transpose) on host.  Layers 0/1 use fp8-e4m3 DoubleRow
projection matmuls (inputs xaug/O0 in fp8); layer 0's hidden state and
recurrent weights are fp8 too.  Layer 2 is all-bf16 (output precision).
Biases are applied via the ScalarE activation bias operand in fp32.
tanh(c) for layers 0/1 via the short divide chain h = o - 2o/(1+e^{2c}).
"""

from contextlib import ExitStack

import numpy as np
import ml_dtypes

import concourse.bacc as bacc
import concourse.tile as tile
from concourse import mybir
from concourse.bass_utils import run_bass_kernel_spmd

F32 = mybir.dt.float32
BF16 = mybir.dt.bfloat16
FP8 = mybir.dt.float8e4
AF = mybir.ActivationFunctionType
OP = mybir.AluOpType
DR = mybir.MatmulPerfMode.DoubleRow

NP_FP8 = mybir.dt.np(FP8)

NCORES = 8
BC = 1024               # batch rows per core
CHUNKS = 2
T = 64
H = 64
NL = 3
D2 = 2 * H              # 128
LN_EPS = 1e-5

SIG_GATES = (0, 1, 3)   # i, f, o  -> sigmoid
TANH_GATE = 2           # g        -> tanh


def _host_prep(x, w_ih, w_hh, b_ih, b_hh, ncores, bc):
    x = np.asarray(x, np.float32)
    w_ih = np.asarray(w_ih, np.float32)
    w_hh = np.asarray(w_hh, np.float32)
    bias = np.asarray(b_ih, np.float32) + np.asarray(b_hh, np.float32)
    t_len = x.shape[1]

    # recurrent weights: layer 0 fp8 (block-diag), layers 1/2 bf16
    rw0 = np.zeros((128, 4, 128), np.float32)
    for g in range(4):
        gs = slice(g * H, (g + 1) * H)
        rw0[0:64, g, 0:64] = w_hh[0, 0, gs, :].T
        rw0[64:128, g, 64:128] = w_hh[0, 1, gs, :].T
    rw0 = rw0.astype(NP_FP8)

    rw12 = np.zeros((128, 2, 4, 128), np.float32)
    for l in (1, 2):
        for g in range(4):
            gs = slice(g * H, (g + 1) * H)
            rw12[0:64, l - 1, g, 0:64] = w_hh[l, 0, gs, :].T
            rw12[64:128, l - 1, g, 64:128] = w_hh[l, 1, gs, :].T
    rw12 = rw12.astype(ml_dtypes.bfloat16)

    # fp8 DoubleRow projection weights for layers 0 and 1: one full-width
    # matmul per gate.  rhs block0 = bwd time slice, block1 = fwd slice;
    # out cols 0:64 (fwd) weighted only in block1, cols 64:128 (bwd) only
    # in block0 (zeros elsewhere).
    l0wdr = np.zeros((9, 2, 4, 128), np.float32)
    for g in range(4):
        gs = slice(g * H, (g + 1) * H)
        l0wdr[0:8, 1, g, 0:64] = w_ih[0, 0, gs, 0:8].T    # fwd -> block1
        l0wdr[0:8, 0, g, 64:128] = w_ih[0, 1, gs, 0:8].T  # bwd -> block0
    l0wdr = l0wdr.astype(NP_FP8)

    pw1dr = np.zeros((128, 2, 4, 128), np.float32)
    for g in range(4):
        gs = slice(g * H, (g + 1) * H)
        pw1dr[:, 1, g, 0:64] = w_ih[1, 0, gs, :].T        # fwd -> block1
        pw1dr[:, 0, g, 64:128] = w_ih[1, 1, gs, :].T      # bwd -> block0
    pw1dr = pw1dr.astype(NP_FP8)

    pw2 = np.zeros((128, 4, 2, 64), np.float32)
    for g in range(4):
        gs = slice(g * H, (g + 1) * H)
        for d in range(2):
            pw2[:, g, d, :] = w_ih[2, d, gs, :].T
    pw2 = pw2.astype(ml_dtypes.bfloat16)

    # biases (fp32, applied via ACT bias operand), all layers
    br = np.zeros((128, NL * 3), np.float32)
    for l in range(NL):
        for j, g in enumerate(SIG_GATES):
            gs = slice(g * H, (g + 1) * H)
            br[0:64, l * 3 + j] = bias[l, 0, gs]
            br[64:128, l * 3 + j] = bias[l, 1, gs]

    gb = np.zeros((128, NL), np.float32)
    gs = slice(TANH_GATE * H, (TANH_GATE + 1) * H)
    for l in range(NL):
        gb[0:64, l] = bias[l, 0, gs]
        gb[64:128, l] = bias[l, 1, gs]

    # Per-core transposed-augmented input xaug[k, t, b] in fp8
    xaug_cores = []
    for c in range(ncores):
        xc = x[c * bc:(c + 1) * bc]              # (bc, T, 8)
        xa = np.empty((9, t_len, bc), np.float32)
        xa[0:8] = xc.transpose(2, 1, 0)
        xa[8] = 1.0
        xaug_cores.append(xa.astype(NP_FP8))

    shared = dict(rw0=rw0, rw12=rw12, l0wdr=l0wdr, pw1dr=pw1dr, pw2=pw2,
                  br=br, gb=gb)
    return shared, xaug_cores


def _emit(nc, tc, ctx, D, bc, t_len):
    bk = bc // CHUNKS

    sbC = ctx.enter_context(tc.tile_pool(name="consts", bufs=1))
    sbA = ctx.enter_context(tc.tile_pool(name="workA", bufs=3))
    sbB = ctx.enter_context(tc.tile_pool(name="workB", bufs=2))
    sbS = ctx.enter_context(tc.tile_pool(name="state", bufs=1))
    ps = ctx.enter_context(tc.tile_pool(name="ps", bufs=1, space="PSUM"))

    def const_tile(shape, dtype, key):
        t = sbC.tile(shape, dtype, name=f"c_{key}", tag=f"c_{key}")
        nc.sync.dma_start(out=t, in_=D[key])
        return t

    rw0_sb = const_tile([128, 4, 128], FP8, "rw0")
    rw12_sb = const_tile([128, 2, 4, 128], BF16, "rw12")
    l0wdr_sb = const_tile([9, 2, 4, 128], FP8, "l0wdr")
    pw1dr_sb = const_tile([128, 2, 4, 128], FP8, "pw1dr")
    pw2_sb = const_tile([128, 4, 2, 64], BF16, "pw2")
    br_sb = const_tile([128, NL * 3], F32, "br")
    gb_sb = const_tile([128, NL], F32, "gb")
    qc_sb = sbC.tile([128, 1], F32)
    nc.vector.memset(qc_sb, float(np.exp(-2.0)))
    neg1_sb = sbC.tile([128, 1], F32)
    nc.vector.memset(neg1_sb, -1.0)

    O = [D[f"o{i}"] for i in range(NL)]
    xaug = D["xaug"]

    h_prev = [None] * CHUNKS
    c_st = [None] * CHUNKS

    def issue_inp(cc, l, k0, nt):
        """Load nt(=2) consecutive timesteps, both directions, into one
        tile: flat [bwd-t0, bwd-t1, fwd-t0, fwd-t1]."""
        c0 = cc * bk
        cols = slice(c0, c0 + bk)
        rthi = t_len - k0
        rtlo = rthi - nt
        src = xaug if l == 0 else O[l - 1]
        p = 9 if l == 0 else 128
        dt = FP8 if l < 2 else BF16
        inp = sbA.tile([p, 2 * nt, bk], dt, tag=f"in{cc}", bufs=3,
                       name="inp")
        nc.sync.dma_start(out=inp[:, 0:nt, :], in_=src[:, rtlo:rthi, cols])
        nc.sync.dma_start(out=inp[:, nt:2 * nt, :],
                          in_=src[:, k0:k0 + nt, cols])
        return inp

    def lstm_mms(cc, l, k, inp, j, rt_j):
        """Projection matmuls (fp8 DoubleRow for l<2), then recurrent."""
        P_ifo = ps.tile([128, 3, bk], F32, tag=f"pifo{cc}")
        P_g = ps.tile([128, bk], F32, tag=f"pg{cc}")
        targets = [(P_ifo[:, 0, :], 0), (P_ifo[:, 1, :], 1),
                   (P_g, TANH_GATE), (P_ifo[:, 2, :], 3)]
        if l < 2:
            w = l0wdr_sb if l == 0 else pw1dr_sb
            # rhs pairs (bwd @ rt_j, fwd @ j) from the flat
            # [bwd-t0, bwd-t1, fwd-t0, fwd-t1] tile; NT=2
            if j == 0:
                rhs = inp[:, 1:3, :]        # (bwd-t1, fwd-t0)
            else:
                rhs = inp[:, 0:4:3, :]      # (bwd-t0, fwd-t1)
            for out_ap, g in targets:
                nc.tensor.matmul(out_ap, w[:, :, g, :], rhs, start=True,
                                 stop=(k == 0), perf_mode=DR,
                                 skip_group_check=True)
        else:
            for out_ap, g in targets:
                nc.tensor.matmul(out_ap[0:64, :], pw2_sb[:, g, 0, :],
                                 inp[:, 2 + j, :], start=True, stop=(k == 0),
                                 tile_position=(0, 0), skip_group_check=True)
                nc.tensor.matmul(out_ap[64:128, :], pw2_sb[:, g, 1, :],
                                 inp[:, rt_j, :], start=True,
                                 stop=(k == 0), tile_position=(0, 64),
                                 skip_group_check=True)
        if k > 0:
            rw = rw0_sb if l == 0 else rw12_sb[:, l - 1]
            # g first: its activation can start while i/f/o recs run
            for out_ap, g in (targets[2], targets[0], targets[1],
                              targets[3]):
                nc.tensor.matmul(out_ap, rw[:, g, :], h_prev[cc],
                                 start=False, stop=True,
                                 skip_group_check=True)
        return P_ifo, P_g

    def lstm_act(cc, l, k, P_ifo, P_g):
        S_ifo = sbB.tile([128, 3, bk], BF16, tag=f"sifo{cc}", bufs=3)
        S_g = sbB.tile([128, bk], BF16, tag=f"sg{cc}")
        idx = l * 3
        nc.scalar.activation(out=S_g, in_=P_g, func=AF.Tanh,
                             bias=gb_sb[:, l:l + 1])
        nc.scalar.activation(out=S_ifo[:, 0, :], in_=P_ifo[:, 0, :],
                             func=AF.Sigmoid, bias=br_sb[:, idx:idx + 1])
        nc.scalar.activation(out=S_ifo[:, 1, :], in_=P_ifo[:, 1, :],
                             func=AF.Sigmoid,
                             bias=br_sb[:, idx + 1:idx + 2])
        nc.scalar.activation(out=S_ifo[:, 2, :], in_=P_ifo[:, 2, :],
                             func=AF.Sigmoid,
                             bias=br_sb[:, idx + 2:idx + 3])

        # c-chain: c bf16 accumulator
        if k == 0:
            c = sbS.tile([128, bk], BF16, tag=f"c{cc}")
            c_st[cc] = c
            nc.vector.tensor_tensor(c, S_ifo[:, 0, :], S_g, op=OP.mult)
        else:
            c = c_st[cc]
            u = sbB.tile([128, bk], BF16, tag=f"u{cc}")
            nc.vector.tensor_tensor(u, S_ifo[:, 0, :], S_g, op=OP.mult)
            v = sbB.tile([128, bk], BF16, tag=f"v{cc}")
            nc.vector.tensor_tensor(v, S_ifo[:, 1, :], c, op=OP.mult)
            nc.vector.tensor_tensor(c, u, v, op=OP.add)
        return S_ifo, c

    def lstm_tail(cc, l, k, S_ifo, c, hsh):
        """tanh(c): short divide chain for layers 0/1; exact ScalarE tanh
        for layer 2 (feeds the output)."""
        h = hsh[:, cc * bk:(cc + 1) * bk]
        if l == NL - 1:
            Tc = sbB.tile([128, bk], BF16, tag=f"tc{cc}")
            nc.scalar.activation(out=Tc, in_=c, func=AF.Tanh)
            nc.vector.tensor_tensor(h, Tc, S_ifo[:, 2, :], op=OP.mult)
        else:
            # h = o * tanh(c) via pow chain (tanh(c) = 2/(1+e^{-2c}) - 1)
            tp = sbB.tile([128, bk], BF16, tag=f"tp{cc}")
            nc.gpsimd.tensor_tensor(tp, qc_sb.broadcast_to([128, bk]), c,
                                    op=OP.pow)
            ut = sbB.tile([128, bk], BF16, tag=f"ut{cc}")
            nc.vector.tensor_scalar(ut, tp, 1.0, 1.0, op0=OP.mult, op1=OP.add)
            r = sbB.tile([128, bk], BF16, tag=f"r{cc}")
            nc.gpsimd.tensor_tensor(r, ut, neg1_sb.broadcast_to([128, bk]),
                                    op=OP.pow)
            w2 = sbB.tile([128, bk], BF16, tag=f"w2{cc}")
            nc.vector.tensor_scalar(w2, r, 2.0, -1.0, op0=OP.mult, op1=OP.add)
            nc.vector.tensor_tensor(h, w2, S_ifo[:, 2, :], op=OP.mult)
        h_prev[cc] = h

    # ---------------- main schedule ----------------
    NT = 2  # load batching in steps
    for l in range(NL):
        pend = {}
        for cc in range(CHUNKS):
            pend[(cc, 0)] = issue_inp(cc, l, 0, NT)
            pend[(cc, 1)] = issue_inp(cc, l, NT, NT)
        hdt = FP8 if l == 0 else BF16
        for k in range(t_len):
            j = k % NT
            w = k // NT
            ph1 = {}
            hsh = sbA.tile([128, bc], hdt, tag="hsh", bufs=3, name="hsh")
            for cc in range(CHUNKS):
                if j == 0 and (w + 3) * NT <= t_len:
                    pend[(cc, w + 2)] = issue_inp(cc, l, (w + 2) * NT, NT)
                inp = pend[(cc, w)]
                rt_j = NT - 1 - j
                mm = lstm_mms(cc, l, k, inp, j, rt_j)
                ph1[cc] = mm
                if j == NT - 1:
                    del pend[(cc, w)]
            for cc in range(CHUNKS):
                P_ifo, P_g = ph1[cc]
                S_ifo, c = lstm_act(cc, l, k, P_ifo, P_g)
                ph1[cc] = (S_ifo, c)
            for cc in range(CHUNKS):
                S_ifo, c = ph1[cc]
                lstm_tail(cc, l, k, S_ifo, c, hsh)
            # chunk-merged stores: fwd halves at t=k, bwd halves at t=rt
            rt = t_len - 1 - k
            nc.sync.dma_start(out=O[l][0:64, k, :], in_=hsh[0:64, :])
            nc.sync.dma_start(out=O[l][64:128, rt, :], in_=hsh[64:128, :])


def build(bc=BC, t_len=T, num_devices=NCORES):
    nc = bacc.Bacc("TRN2", target_bir_lowering=False, debug=False,
                   num_devices=num_devices)
    D = {}

    def inp(name, shape, dtype=F32):
        D[name] = nc.dram_tensor(name, shape, dtype, kind="ExternalInput").ap()

    inp("xaug", [9, t_len, bc], FP8)
    inp("rw0", [128, 4, 128], FP8)
    inp("rw12", [128, 2, 4, 128], BF16)
    inp("l0wdr", [9, 2, 4, 2, 64], FP8)
    inp("pw1dr", [128, 2, 4, 2, 64], FP8)
    inp("pw2", [128, 4, 2, 64], BF16)
    inp("br", [128, NL * 3])
    inp("gb", [128, NL])
    D["o0"] = nc.dram_tensor("o0", [128, t_len, bc], FP8).ap()
    D["o1"] = nc.dram_tensor("o1", [128, t_len, bc], BF16).ap()
    D["o2"] = nc.dram_tensor(
        "o2", [128, t_len, bc], BF16, kind="ExternalOutput").ap()

    with tile.TileContext(nc) as tc:
        with ExitStack() as ctx:
            _emit(nc, tc, ctx, D, bc, t_len)
    nc.compile()
    return nc


_BUILD_CACHE = {}


def kernel(x, w_ih, w_hh, b_ih, b_hh, w_res, b_res, ln_gamma, ln_beta):
    x = np.asarray(x, np.float32)
    ln_gamma = np.asarray(ln_gamma, np.float32)
    ln_beta = np.asarray(ln_beta, np.float32)
    w_res = np.asarray(w_res, np.float32)
    b_res = np.asarray(b_res, np.float32)

    shared, xaug_cores = _host_prep(
        x, w_ih, w_hh, b_ih, b_hh, NCORES, BC)
    if "nc" not in _BUILD_CACHE:
        _BUILD_CACHE["nc"] = build()
    nc = _BUILD_CACHE["nc"]

    in_maps = []
    for c in range(NCORES):
        m = dict(shared)
        m["xaug"] = xaug_cores[c]
        in_maps.append(m)

    res = run_bass_kernel_spmd(nc, in_maps, core_ids=list(range(NCORES)))
    outs = []
    for c in range(NCORES):
        o2 = res.results[c]["o2"]                # [128, T, bc] bf16
        outs.append(np.asarray(o2).transpose(2, 1, 0))  # (bc, T, 128)
    lstm_out = np.concatenate(outs, axis=0).astype(np.float32)

    # host-side final stage: relu + residual + layernorm
    np.maximum(lstm_out, 0.0, out=lstm_out)
    out = lstm_out
    out += x @ w_res.T + b_res
    mu = out.mean(axis=-1, keepdims=True, dtype=np.float32)
    out -= mu
    var = np.einsum('btd,btd->bt', out, out,
                    dtype=np.float32) / out.shape[-1]
    out *= (1.0 / np.sqrt(var + LN_EPS))[..., None]
    out *= ln_gamma
    out += ln_beta
    return np.ascontiguousarray(out.astype(np.float32))
